# revision 1
# baseline (speedup 1.0000x reference)
"""Vocab-parallel projection + cross-entropy loss kernel for TRN2 (8 NeuronCores).

Problem: x [2,2048,2048] f32, y [2,2048] int64, W [128000,2048] f32
  loss = mean_n( logsumexp_v(x_n . W_v) - x_n . W_{y_n} )

Sharding (8 cores):
  - W's vocab dim split 8 ways (16000 rows/core): each core computes
    out_s[n] = sum_{v in shard} exp(logit[n, v]) for all 4096 tokens.
    (No max subtraction needed: logits ~ N(0, 1/3).)
  - tokens split 8 ways for the true-logit term: core c receives
    xy = x rows and wy = W[y] rows for its 512 tokens and computes
    out_t[j] = xy[j] . wy[j] on VectorE.
Host combine: loss = mean(log(sum_i out_s_i) - concat_i out_t_i).

Per-core device kernel (fp8 path):
  - W shard: SWDGE cast-DMA f32->bf16 into DRAM, XBAR transpose-load
    [h x v] bf16 slabs, VectorE scale(x64)+cast to fp8e4
  - x: HWDGE load + VectorE cast to bf16 DRAM, XBAR transpose-load,
    VectorE scale(x32)+cast to fp8e4 (x^T resident in SBUF)
  - per vocab tile (512): 8 DoubleRow fp8 matmuls per 128-token block
    accumulate [128tok x 512v] logits*2048 in PSUM; one ScalarE Exp with
    scale=1/2048 and accum_out -> per-(block,tile) partial sums
"""

import numpy as np

B, S, H, V = 2, 2048, 2048, 128000
N_CORES = 8
N_TOK = B * S                 # 4096
V_SHARD = V // N_CORES        # 16000
TOK_SHARD = N_TOK // N_CORES  # 512
P = 128
V_TILE = 512                  # one PSUM bank of f32
X_SCALE = 32.0
W_SCALE = 64.0

_KERNEL_CACHE = {}


def _build(n_tok, h, vsh, tok_sh, use_fp8=True, debug=False, do_true=True, do_main=True):
    """Build + compile the single-core SPMD Bass program."""
    import concourse.mybir as mybir
    import concourse.tile as tile
    from concourse import bacc

    kt = h // P                       # k-tiles over hidden dim
    n_tb = n_tok // P                 # token blocks
    v_sizes = [V_TILE] * (vsh // V_TILE)
    if vsh % V_TILE:
        v_sizes.append(vsh % V_TILE)  # remainder must be multiple of 16 (XBAR)
    n_vt = len(v_sizes)
    descale = 1.0 / (X_SCALE * W_SCALE) if use_fp8 else 1.0

    nc = bacc.Bacc("TRN2", target_bir_lowering=False, debug=debug)
    f32 = mybir.dt.float32
    bf16 = mybir.dt.bfloat16
    fp8 = mybir.dt.float8e4
    mm_dt = fp8 if use_fp8 else bf16

    x_in = nc.dram_tensor("x", [n_tok, h], f32, kind="ExternalInput")
    w_in = nc.dram_tensor("w", [vsh, h], f32, kind="ExternalInput")
    xy_in = nc.dram_tensor("xy", [tok_sh, h], f32, kind="ExternalInput")
    wy_in = nc.dram_tensor("wy", [tok_sh, h], f32, kind="ExternalInput")
    out_s = nc.dram_tensor("out_s", [n_tok], f32, kind="ExternalOutput")
    out_t = nc.dram_tensor("out_t", [tok_sh], f32, kind="ExternalOutput")

    xb = nc.dram_tensor("xb", [n_tok, h], bf16)      # bf16 copy of x
    wb = nc.dram_tensor("wb", [vsh, h], bf16)        # bf16 copy of W shard

    with tile.TileContext(nc) as tc:
        with (
            tc.tile_pool(name="const", bufs=1) as cpool,
            tc.tile_pool(name="wslab", bufs=3) as wpool,
            tc.tile_pool(name="w8p", bufs=2) as w8pool,
            tc.tile_pool(name="psum", bufs=8, space="PSUM") as ppool,
            tc.tile_pool(name="gath", bufs=1) as gpool,
            tc.tile_pool(name="xrow", bufs=1) as xpool,
            tc.tile_pool(name="junk", bufs=1) as jpool,
            tc.tile_pool(name="stage", bufs=3) as stpool,
            tc.tile_pool(name="castp", bufs=2) as ctpool,
            tc.tile_pool(name="xtmp", bufs=2) as xtpool,
        ):
            # ---- persistent SBUF tensors ----
            xT = cpool.tile([P, kt, n_tok], mm_dt, tag="xT")
            sacc = cpool.tile([P, n_tb, n_vt], f32, tag="sacc")
            tacc = cpool.tile([P, tok_sh // P], f32, tag="tacc")
            s2 = cpool.tile([P, n_tb], f32, tag="s2")

            # ---- phase T: true logits for this core's token slice ----
            for c in range(tok_sh // P if do_true else 0):
                wy = gpool.tile([P, h], f32, tag="wy")
                nc.sync.dma_start(wy[:], wy_in[c * P : (c + 1) * P, :])
                xf = xpool.tile([P, h], f32, tag="xf")
                nc.sync.dma_start(xf[:], xy_in[c * P : (c + 1) * P, :])
                junk = jpool.tile([P, h], f32, tag="junk")
                nc.vector.tensor_tensor(
                    out=junk[:], in0=xf[:], in1=wy[:], op=mybir.AluOpType.mult
                )
                nc.vector.tensor_reduce(
                    out=tacc[:, c : c + 1],
                    in_=junk[:],
                    axis=mybir.AxisListType.X,
                    op=mybir.AluOpType.add,
                )
            if do_true:
                nc.sync.dma_start(out_t[:].rearrange("(a b) -> b a", b=P), tacc[:])

            if do_main:
                # ---- phase 0: x -> bf16 -> x^T -> mm dtype, in row halves ----
                # loads stream on the sync queue; stores + XBAR transposes share
                # the scalar queue (store(rb) paces at DVE speed, which is fine
                # since transposes of a half follow all of its stores anyway)
                n_half = n_tok // 2
                rb_half = n_half // P
                for half in range(2):
                    for rbh in range(rb_half):
                        rb = half * rb_half + rbh
                        stage = stpool.tile([P, h], f32, tag="stage")
                        nc.sync.dma_start(stage[:], x_in[rb * P : (rb + 1) * P, :])
                        cast = ctpool.tile([P, h], bf16, tag="cast")
                        nc.vector.tensor_copy(out=cast[:], in_=stage[:])
                        nc.scalar.dma_start(xb[rb * P : (rb + 1) * P, :], cast[:])
                    for k in range(kt):
                        if use_fp8:
                            xtmp = xtpool.tile([P, n_half], bf16, tag="xtmp")
                            nc.sync.dma_start_transpose(
                                xtmp[:],
                                xb[half * n_half : (half + 1) * n_half, k * P : (k + 1) * P],
                            )
                            nc.vector.tensor_scalar_mul(
                                xT[:, k, half * n_half : (half + 1) * n_half],
                                xtmp[:],
                                X_SCALE,
                            )
                        else:
                            nc.sync.dma_start_transpose(
                                xT[:, k, half * n_half : (half + 1) * n_half],
                                xb[half * n_half : (half + 1) * n_half, k * P : (k + 1) * P],
                            )

            # ---- phase 1: main matmul + exp loop ----
            v0 = 0
            for vt, vsz in enumerate(v_sizes if do_main else []):
                # W rows -> bf16 via SWDGE cast-DMA (DRAM->DRAM), split in two
                vh = vsz // 2
                nc.gpsimd.dma_start(wb[v0 : v0 + vh, :], w_in[v0 : v0 + vh, :])
                nc.gpsimd.dma_start(wb[v0 + vh : v0 + vsz, :], w_in[v0 + vh : v0 + vsz, :])
                wslab = wpool.tile([P, kt, V_TILE], bf16, tag="wslab")
                for k in range(kt):
                    nc.sync.dma_start_transpose(
                        wslab[:, k, :vsz], wb[v0 : v0 + vsz, k * P : (k + 1) * P]
                    )
                if use_fp8:
                    w8 = w8pool.tile([P, kt, V_TILE], fp8, tag="w8")
                    nc.vector.tensor_scalar_mul(w8[:], wslab[:], W_SCALE)
                    rhs_slab = w8
                else:
                    rhs_slab = wslab
                for tb in range(n_tb):
                    psum = ppool.tile([P, V_TILE], f32, tag="psum")
                    if use_fp8:
                        for kk in range(0, kt, 2):
                            nc.tensor.matmul(
                                psum[:, :vsz],
                                lhsT=xT[:, kk : kk + 2, tb * P : (tb + 1) * P],
                                rhs=rhs_slab[:, kk : kk + 2, :vsz],
                                start=(kk == 0),
                                stop=(kk == kt - 2),
                                perf_mode=mybir.MatmulPerfMode.DoubleRow,
                            )
                    else:
                        for k in range(kt):
                            nc.tensor.matmul(
                                psum[:, :vsz],
                                lhsT=xT[:, k, tb * P : (tb + 1) * P],
                                rhs=rhs_slab[:, k, :vsz],
                                start=(k == 0),
                                stop=(k == kt - 1),
                            )
                    # exp(descale * psum) in place, free-dim sum -> sacc
                    nc.scalar.activation(
                        out=psum[:, :vsz],
                        in_=psum[:, :vsz],
                        func=mybir.ActivationFunctionType.Exp,
                        scale=descale,
                        accum_out=sacc[:, tb, vt : vt + 1],
                    )
                v0 += vsz

            # ---- phase 2: finalize s ----
            if do_main:
                nc.vector.tensor_reduce(
                    out=s2[:], in_=sacc[:], axis=mybir.AxisListType.X, op=mybir.AluOpType.add
                )
                nc.sync.dma_start(out_s[:].rearrange("(a b) -> b a", b=P), s2[:])

    nc.compile()
    return nc


def _get_kernel(n_tok, h, vsh, tok_sh):
    key = (n_tok, h, vsh, tok_sh)
    if key not in _KERNEL_CACHE:
        _KERNEL_CACHE[key] = _build(n_tok, h, vsh, tok_sh)
    return _KERNEL_CACHE[key]


def make_in_maps(x, y, W, n_cores=N_CORES):
    """Shard full inputs into per-core input maps."""
    n_tok = x.reshape(-1, x.shape[-1]).shape[0]
    h = x.shape[-1]
    v = W.shape[0]
    vsh = v // n_cores
    tok_sh = n_tok // n_cores
    xf = np.ascontiguousarray(x.reshape(n_tok, h), dtype=np.float32)
    yf = y.reshape(n_tok)
    wy_full = np.ascontiguousarray(W[yf], dtype=np.float32)  # [n_tok, h]
    in_maps = []
    for c in range(n_cores):
        lo, hi = c * vsh, (c + 1) * vsh
        t0, t1 = c * tok_sh, (c + 1) * tok_sh
        in_maps.append(
            {
                "x": xf,
                "w": np.ascontiguousarray(W[lo:hi], dtype=np.float32),
                "xy": np.ascontiguousarray(xf[t0:t1]),
                "wy": np.ascontiguousarray(wy_full[t0:t1]),
            }
        )
    return in_maps


def combine(results):
    """Host-side unshard: reduce per-core partials to the scalar loss."""
    s = np.sum([r["out_s"].astype(np.float64) for r in results], axis=0)
    t = np.concatenate([r["out_t"].astype(np.float64) for r in results])
    return np.float32(np.mean(np.log(s) - t))


def run_sharded(x, y, W, trace=False):
    from concourse.bass_utils import run_bass_kernel_spmd

    n_tok = x.reshape(-1, x.shape[-1]).shape[0]
    h = x.shape[-1]
    vsh = W.shape[0] // N_CORES
    nc = _get_kernel(n_tok, h, vsh, n_tok // N_CORES)
    in_maps = make_in_maps(x, y, W)
    res = run_bass_kernel_spmd(nc, in_maps, list(range(N_CORES)), trace=trace)
    return res


def kernel(x, y, W):
    res = run_sharded(np.asarray(x), np.asarray(y), np.asarray(W))
    return combine(res.results)



# revision 2
# speedup vs baseline: 1.0962x; 1.0962x over previous
"""Vocab-parallel projection + cross-entropy loss kernel for TRN2 (8 NeuronCores).

Problem: x [2,2048,2048] f32, y [2,2048] int64, W [128000,2048] f32
  loss = mean_n( logsumexp_v(x_n . W_v) - x_n . W_{y_n} )

Sharding (8 cores):
  - W's vocab dim split 8 ways (16000 rows/core): each core computes
    out_s[n] = sum_{v in shard} exp(logit[n, v]) for all 4096 tokens.
    (No max subtraction needed: logits ~ N(0, 1/3).)
  - tokens split 8 ways for the true-logit term: core c receives
    xy = x rows and wy = W[y] rows for its 512 tokens and computes
    out_t[j] = xy[j] . wy[j] on VectorE.
Host combine: loss = mean(log(sum_i out_s_i) - concat_i out_t_i).

Per-core device kernel (v2, fp8):
  - x: f32 slab loads alternating across the two HWDGE queues, DVE cast
    to bf16, PE transpose (identity matmul) into PSUM, DVE scale(x32)
    cast to fp8e4 into resident xT. No DRAM roundtrip, no XBAR use.
  - W shard: SWDGE cast-DMA f32->bf16 into a 2-slot DRAM ring (paced by
    WAR deps so the casts can't flood the DMA rings at t=0), XBAR
    transpose-loads split across both HWDGE queues, DVE scale(x64) cast
    to fp8e4.
  - main loop: per vocab tile (512) x token block (128), 8 DoubleRow
    fp8 matmuls accumulate logits*2048 in PSUM; ScalarE Exp with
    scale=1/2048 and accum_out -> per-(block,tile) partial sums.
  - vt0's matmuls are interleaved into the x pipeline on the PE queue so
    the PE is busy from ~25us instead of ~365us.
"""

import numpy as np

B, S, H, V = 2, 2048, 2048, 128000
N_CORES = 8
N_TOK = B * S                 # 4096
V_SHARD = V // N_CORES        # 16000
TOK_SHARD = N_TOK // N_CORES  # 512
P = 128
V_TILE = 512                  # one PSUM bank of f32
X_SCALE = 32.0
W_SCALE = 64.0
WB_RING = 2                   # DRAM staging slots for W bf16 cast

_KERNEL_CACHE = {}


def _build(n_tok, h, vsh, tok_sh, debug=False):
    """Build + compile the single-core SPMD Bass program."""
    import concourse.mybir as mybir
    import concourse.tile as tile
    from concourse import bacc, masks

    kt = h // P                       # k-tiles over hidden dim
    n_tb = n_tok // P                 # token blocks
    v_sizes = [V_TILE] * (vsh // V_TILE)
    if vsh % V_TILE:
        v_sizes.append(vsh % V_TILE)  # remainder must be multiple of 16 (XBAR)
    n_vt = len(v_sizes)
    descale = 1.0 / (X_SCALE * W_SCALE)

    nc = bacc.Bacc("TRN2", target_bir_lowering=False, debug=debug)
    f32 = mybir.dt.float32
    bf16 = mybir.dt.bfloat16
    fp8 = mybir.dt.float8e4

    x_in = nc.dram_tensor("x", [n_tok, h], f32, kind="ExternalInput")
    w_in = nc.dram_tensor("w", [vsh, h], f32, kind="ExternalInput")
    xy_in = nc.dram_tensor("xy", [tok_sh, h], f32, kind="ExternalInput")
    wy_in = nc.dram_tensor("wy", [tok_sh, h], f32, kind="ExternalInput")
    out_s = nc.dram_tensor("out_s", [n_tok], f32, kind="ExternalOutput")
    out_t = nc.dram_tensor("out_t", [tok_sh], f32, kind="ExternalOutput")

    # W bf16 staging ring in DRAM; slot reuse creates WAR deps that pace
    # the SWDGE casts against the XBAR transpose-loads.
    wb = nc.dram_tensor("wb", [WB_RING, V_TILE, h], bf16)

    hw_q = None  # set inside ctx: [nc.sync, nc.scalar]

    with tile.TileContext(nc) as tc:
        with (
            tc.tile_pool(name="const", bufs=1) as cpool,
            tc.tile_pool(name="xstage", bufs=3) as xspool,
            tc.tile_pool(name="xcast", bufs=3) as xcpool,
            tc.tile_pool(name="trp", bufs=2, space="PSUM") as trpool,
            tc.tile_pool(name="wslab", bufs=3) as wpool,
            tc.tile_pool(name="w8p", bufs=2) as w8pool,
            tc.tile_pool(name="psum", bufs=5, space="PSUM") as ppool,
            tc.tile_pool(name="gath", bufs=1) as gpool,
            tc.tile_pool(name="xrow", bufs=1) as xpool,
            tc.tile_pool(name="junk", bufs=1) as jpool,
        ):
            hw_q = [nc.sync, nc.scalar]

            # ---- persistent SBUF tensors ----
            xT = cpool.tile([P, kt, n_tok], fp8, tag="xT")
            sacc = cpool.tile([P, n_tb, n_vt], f32, tag="sacc")
            tacc = cpool.tile([P, tok_sh // P], f32, tag="tacc")
            s2 = cpool.tile([P, n_tb], f32, tag="s2")
            ident = cpool.tile([P, P], bf16, tag="ident")
            masks.make_identity(nc, ident[:])

            # ---- W pipeline stage (emitted per vt from the driver below) ----
            def w_prep(vt, vsz, v0):
                slot = vt % WB_RING
                vh = vsz // 2
                nc.gpsimd.dma_start(wb[slot, :vh, :], w_in[v0 : v0 + vh, :])
                nc.gpsimd.dma_start(
                    wb[slot, vh:vsz, :], w_in[v0 + vh : v0 + vsz, :]
                )
                wslab = wpool.tile([P, kt, V_TILE], bf16, tag="wslab")
                for k in range(kt):
                    hw_q[k % 2].dma_start_transpose(
                        wslab[:, k, :vsz], wb[slot, :vsz, k * P : (k + 1) * P]
                    )
                w8 = w8pool.tile([P, kt, V_TILE], fp8, tag="w8")
                nc.vector.tensor_scalar_mul(w8[:], wslab[:], W_SCALE)
                return w8

            def mm_tile(w8, vt, vsz, tb):
                psum = ppool.tile([P, V_TILE], f32, tag="psum")
                for kk in range(0, kt, 2):
                    nc.tensor.matmul(
                        psum[:, :vsz],
                        lhsT=xT[:, kk : kk + 2, tb * P : (tb + 1) * P],
                        rhs=w8[:, kk : kk + 2, :vsz],
                        start=(kk == 0),
                        stop=(kk == kt - 2),
                        perf_mode=mybir.MatmulPerfMode.DoubleRow,
                    )
                nc.scalar.activation(
                    out=psum[:, :vsz],
                    in_=psum[:, :vsz],
                    func=mybir.ActivationFunctionType.Exp,
                    scale=descale,
                    accum_out=sacc[:, tb, vt : vt + 1],
                )

            # ---- x pipeline with vt0/vt1 W prep + vt0 matmuls interleaved ----
            # PE queue order: [tr(0..7)..., (tr(j), mm(vt0, j-8))..., mm rest]
            w8_0 = w8_1 = None
            for tb in range(n_tb):
                if tb == 2:
                    w8_0 = w_prep(0, v_sizes[0], 0)
                if tb == 18:
                    w8_1 = w_prep(1, v_sizes[1], v_sizes[0])
                xf = xspool.tile([P, h], f32, tag="xf")
                hw_q[tb % 2].dma_start(xf[:], x_in[tb * P : (tb + 1) * P, :])
                xc = xcpool.tile([P, h], bf16, tag="xc")
                nc.vector.tensor_copy(out=xc[:], in_=xf[:])
                for kg in range(2):  # two PSUM banks of 8 transposed blocks
                    trp = trpool.tile([P, 8, P], bf16, tag="trp")
                    for j in range(8):
                        k = kg * 8 + j
                        nc.tensor.transpose(
                            trp[:, j, :], xc[:, k * P : (k + 1) * P], ident[:]
                        )
                    nc.vector.tensor_scalar_mul(
                        xT[:, kg * 8 : (kg + 1) * 8, tb * P : (tb + 1) * P],
                        trp[:],
                        X_SCALE,
                    )
                if tb >= 8:
                    mm_tile(w8_0, 0, v_sizes[0], tb - 8)

            for tb in range(n_tb - 8, n_tb):
                mm_tile(w8_0, 0, v_sizes[0], tb)

            # ---- main loop over remaining vocab tiles ----
            v0 = v_sizes[0] + v_sizes[1]
            for vt in range(2, n_vt):
                vsz = v_sizes[vt]
                w8_next = w_prep(vt, vsz, v0) if vt < n_vt else None
                # run vt-1's matmuls (w8 from previous stage)
                w8_cur = w8_1 if vt == 2 else w8_prev  # noqa: F821
                for tb in range(n_tb):
                    mm_tile(w8_cur, vt - 1, v_sizes[vt - 1], tb)
                w8_prev = w8_next
                v0 += vsz
            for tb in range(n_tb):
                mm_tile(w8_prev, n_vt - 1, v_sizes[n_vt - 1], tb)

            # ---- phase T: true logits for this core's token slice ----
            for c in range(tok_sh // P):
                wyt = gpool.tile([P, h], f32, tag="wy")
                hw_q[c % 2].dma_start(wyt[:], wy_in[c * P : (c + 1) * P, :])
                xft = xpool.tile([P, h], f32, tag="xf_t")
                hw_q[(c + 1) % 2].dma_start(xft[:], xy_in[c * P : (c + 1) * P, :])
                junk = jpool.tile([P, h], f32, tag="junk")
                nc.vector.tensor_tensor(
                    out=junk[:], in0=xft[:], in1=wyt[:], op=mybir.AluOpType.mult
                )
                nc.vector.tensor_reduce(
                    out=tacc[:, c : c + 1],
                    in_=junk[:],
                    axis=mybir.AxisListType.X,
                    op=mybir.AluOpType.add,
                )
            nc.sync.dma_start(out_t[:].rearrange("(a b) -> b a", b=P), tacc[:])

            # ---- finalize s ----
            nc.vector.tensor_reduce(
                out=s2[:], in_=sacc[:], axis=mybir.AxisListType.X, op=mybir.AluOpType.add
            )
            nc.sync.dma_start(out_s[:].rearrange("(a b) -> b a", b=P), s2[:])

    nc.compile()
    return nc


def _get_kernel(n_tok, h, vsh, tok_sh):
    key = (n_tok, h, vsh, tok_sh)
    if key not in _KERNEL_CACHE:
        _KERNEL_CACHE[key] = _build(n_tok, h, vsh, tok_sh)
    return _KERNEL_CACHE[key]


def make_in_maps(x, y, W, n_cores=N_CORES):
    """Shard full inputs into per-core input maps."""
    n_tok = x.reshape(-1, x.shape[-1]).shape[0]
    h = x.shape[-1]
    v = W.shape[0]
    vsh = v // n_cores
    tok_sh = n_tok // n_cores
    xf = np.ascontiguousarray(x.reshape(n_tok, h), dtype=np.float32)
    yf = y.reshape(n_tok)
    wy_full = np.ascontiguousarray(W[yf], dtype=np.float32)  # [n_tok, h]
    in_maps = []
    for c in range(n_cores):
        lo, hi = c * vsh, (c + 1) * vsh
        t0, t1 = c * tok_sh, (c + 1) * tok_sh
        in_maps.append(
            {
                "x": xf,
                "w": np.ascontiguousarray(W[lo:hi], dtype=np.float32),
                "xy": np.ascontiguousarray(xf[t0:t1]),
                "wy": np.ascontiguousarray(wy_full[t0:t1]),
            }
        )
    return in_maps


def combine(results):
    """Host-side unshard: reduce per-core partials to the scalar loss."""
    s = np.sum([r["out_s"].astype(np.float64) for r in results], axis=0)
    t = np.concatenate([r["out_t"].astype(np.float64) for r in results])
    return np.float32(np.mean(np.log(s) - t))


def run_sharded(x, y, W, trace=False):
    from concourse.bass_utils import run_bass_kernel_spmd

    n_tok = x.reshape(-1, x.shape[-1]).shape[0]
    h = x.shape[-1]
    vsh = W.shape[0] // N_CORES
    nc = _get_kernel(n_tok, h, vsh, n_tok // N_CORES)
    in_maps = make_in_maps(x, y, W)
    res = run_bass_kernel_spmd(nc, in_maps, list(range(N_CORES)), trace=trace)
    return res


def kernel(x, y, W):
    res = run_sharded(np.asarray(x), np.asarray(y), np.asarray(W))
    return combine(res.results)
